# revision 4
# baseline (speedup 1.0000x reference)
"""Trainium2 Bass kernel for ConvSelfAttention (B=4, C=128, W=H=64).

Reference computation (per batch b, with N = W*H = 4096):
    q = wq @ x + bq ; k = wk @ x + bk ; v = wv @ x + bv        # [C, N]
    S[n, m] = (q[:, n] . k[:, m]) / sqrt(C)
    A = softmax(S, axis=m)                                     # [N, N]
    out[c, n] = sum_m v[c, m] A[n, m]
    y = wo @ out + bo
    result = gamma * y + x

Sharding: 8 cores = 4 batches x 2 halves of the attention-row dim n.
Each core holds full x[b] (for k, v) and computes out[:, n_slice].

v3 architecture (vs the fp16 v1 baseline at 101us):
  - Scores: fp8e4m3 DoubleRow matmuls in a [64, 2, .] channel-split
    layout (contraction 2x64) -- 2x the fp16 column rate (~108ns per
    512-col matmul vs 213).  The scale A8 = 8/ln2 (and 1/sqrt(C)) is
    folded into wq host-side so PSUM scores arrive pre-scaled for the
    fp8 exp bit trick.
  - exp (the engine wall): split between Scalar (native Exp, fp8 out,
    scale=1/A8, bias=-ln SC) and Vector (Schraudolph bit trick:
    int8(round(max(S' + B8, 0))) reinterpreted as fp8e4m3 ~ exp(S)/SC).
    Weights are normalized by their own approximate denominator so
    common-mode error cancels; the residual-dominated output tolerates
    the rest (tolerance 2e-2; attention is ~12% of the output norm).
    Engine choice per job comes from a build-time greedy balancer that
    also accounts for the projection-drain copies.
  - P@V and the denominator (ones-matmul) both run as fp8 DoubleRow
    matmuls (contraction 2x128 per pass, 213ns per superjob each).
    The Pool engine is NOT used for math: measured fp8 tensor ops on
    Pool run 2-4x slower than modeled and it cannot read PSUM.
  - Loop: 4 blocks of 512 n-cols x 16 m-tile-pairs.  PSUM (8 banks):
    scores 2x[128,2,512] (4) + P@V accum [128,512] (1) + den [128,512]
    (1) + a 2-slot ring shared by projection drains -> out-proj (2).

Host-side folding: 1/sqrt(C) and A8 into wq, bq; bk dropped (cancels in
softmax); bv folded into bo via wo (softmax rows sum to 1); gamma into
wo and bo_eff.
"""

import math
import os
import sys

import numpy as np

if "/opt/trn_rl_repo" not in sys.path:
    sys.path.insert(0, "/opt/trn_rl_repo")

B, C, W, H = 4, 128, 64, 64
N = W * H            # 4096
HALF = N // 2        # 2048 n-columns per core
CHUNK = 512
MT = N // 128        # 32 m-tiles
TP = MT // 2         # 16 m-tile pairs
NBLK = HALF // CHUNK  # 4 n-blocks per core

LN2 = math.log(2.0)
A8 = 8.0 / LN2       # folded into wq host-side; scores arrive as A8*S
SC = 32.0            # weights are exp(S)/SC (fp8e4m3 range guard)
B8 = 8.0 * 7.0 - A8 * math.log(SC) - 0.22   # Schraudolph bias (round conv)

# build-time engine cost model (ns) for the greedy exp balancer
COST_EXP_ACT = 1113.0
COST_EXP_DVE = 1119.0
COST_COPY_ACT = 627.0
COST_COPY_DVE = 683.0
COST_EPI_DVE = 3 * 683.0

_BUILT = {}


def _build():
    if "nc" in _BUILT:
        return _BUILT["nc"]

    import concourse.bass as bass
    from concourse import bacc, mybir
    from concourse.tile import TileContext

    f32 = mybir.dt.float32
    f16 = mybir.dt.float16
    f8 = mybir.dt.float8e4
    i8 = mybir.dt.int8
    DR = mybir.MatmulPerfMode.DoubleRow
    ADD = mybir.AluOpType.add
    MAX = mybir.AluOpType.max

    nc = bacc.Bacc("TRN2", target_bir_lowering=False)

    x_d = nc.dram_tensor("x", [C, N], f16, kind="ExternalInput")
    wp_d = nc.dram_tensor("wpack", [C, 4 * C], f16, kind="ExternalInput")
    bp_d = nc.dram_tensor("bpack", [C, 2], f32, kind="ExternalInput")
    y_d = nc.dram_tensor("y", [C, HALF], f32, kind="ExternalOutput")

    busy = {"A": 0.0, "D": 0.0}   # projected ACT / DVE busy-ns

    with TileContext(nc) as tc:
        with (
            tc.tile_pool(name="consts", bufs=1) as consts,
            tc.tile_pool(name="bigs", bufs=1) as bigs,
            tc.tile_pool(name="exps", bufs=6) as exps,
            tc.tile_pool(name="smalls", bufs=2) as smalls,
            tc.tile_pool(name="outs", bufs=3) as outs,
            tc.tile_pool(name="xs", bufs=4) as xs,
            tc.tile_pool(name="ps_s", bufs=2, space="PSUM") as pp_s,
            tc.tile_pool(name="ps_o", bufs=1, space="PSUM") as pp_o,
            tc.tile_pool(name="ps_d", bufs=1, space="PSUM") as pp_d,
            tc.tile_pool(name="ps_x", bufs=2, space="PSUM") as pp_x,
        ):
            # ---- constants & inputs -------------------------------------
            wp_sb = consts.tile([C, 4 * C], f16, tag="wp")
            bp_sb = consts.tile([C, 2], f32, tag="bp")
            ones8 = consts.tile([128, 2, C], f8, tag="on8")
            bias_e = consts.tile([128, 1], f32, tag="be")
            dummy = consts.tile([C, 1], f16, tag="dmy")

            wv_sb = wp_sb[:, 2 * C:3 * C]
            wo_sb = wp_sb[:, 3 * C:4 * C]  # gamma folded
            bo_sb = bp_sb[:, 1:2]

            # Warm the ACT exp table while DMAs are in flight.
            nc.vector.memset(bias_e, -math.log(SC))
            nc.vector.memset(dummy, 0.0)
            nc.scalar.activation(
                dummy, dummy, mybir.ActivationFunctionType.Exp,
                bias=bias_e[:, 0:1],
            )
            nc.vector.memset(ones8, 1.0)

            k8 = bigs.tile([64, 2, N], f8, tag="k8")
            q8 = bigs.tile([64, 2, HALF], f8, tag="q8")
            vT8 = bigs.tile([128, TP, 2, C], f8, tag="vT8")
            outN = bigs.tile([C, HALF], f16, tag="outN")

            # x pieces: 4 x 1024 cols
            nc.sync.dma_start(out=wp_sb, in_=wp_d[:, :])
            nc.gpsimd.dma_start(out=bp_sb, in_=bp_d[:, :])
            piece = []
            for pi in range(4):
                xp = xs.tile([C, 1024], f16, tag="xp", name=f"xp{pi}")
                eng = nc.scalar if pi < 2 else nc.sync
                eng.dma_start(out=xp, in_=x_d[:, bass.ds(1024 * pi, 1024)])
                piece.append(xp)

            def xh(cn, w):
                pi, off = divmod(cn, 1024)
                assert off + w <= 1024
                return piece[pi][:, bass.ds(off, w)]

            # ---- production units (512 m-cols each) ---------------------
            # Channel-split [64, 2, .] layouts: two 64-wide projection
            # matmuls per chunk; both j halves land on partitions 0..63.
            def kunit(c):
                for j in range(2):
                    ps = pp_x.tile([64, CHUNK], f32, tag="x",
                                   name=f"psk{c}_{j}")
                    nc.tensor.matmul(
                        ps, wp_sb[:, bass.ds(C + 64 * j, 64)],
                        xh(c * CHUNK, CHUNK), start=True, stop=True,
                    )
                    nc.scalar.activation(
                        k8[:, j, bass.ds(c * CHUNK, CHUNK)], ps,
                        mybir.ActivationFunctionType.Copy,
                    )
                    busy["A"] += COST_COPY_ACT

            def qunit(c):
                for j in range(2):
                    ps = pp_x.tile([64, CHUNK], f32, tag="x",
                                   name=f"psq{c}_{j}")
                    nc.tensor.matmul(
                        ps, wp_sb[:, bass.ds(64 * j, 64)],
                        xh(c * CHUNK, CHUNK), start=True, stop=True,
                    )
                    nc.vector.tensor_scalar_add(
                        q8[:, j, bass.ds(c * CHUNK, CHUNK)], ps,
                        bp_sb[bass.ds(64 * j, 64), 0:1],
                    )
                    busy["D"] += COST_COPY_DVE

            def vunit(g):
                ps = pp_x.tile([128, CHUNK], f32, tag="x", name=f"psv{g}")
                for tt in range(4):
                    nc.tensor.matmul(
                        ps[:, bass.ts(tt, 128)],
                        xh(g * CHUNK + tt * 128, 128),
                        wv_sb, start=True, stop=True,
                    )
                nc.vector.tensor_copy(
                    vT8[:, bass.ds(2 * g, 2), :, :],
                    ps.rearrange("p (t c) -> p t c", c=C),
                )
                busy["D"] += COST_COPY_DVE

            # prefetch the units the first superjobs need
            kunit(0)
            qunit(0)
            vunit(0)
            # bundles: (pop_at_superjob, fn); k/v chunk c is consumed from
            # superjob 2c on; q chunk c from block c (superjob 16c).
            bundles = []
            for c in range(1, 8):
                bundles.append(lambda c=c: kunit(c))
                bundles.append(lambda g=c: vunit(g))
            bundles.append(lambda: qunit(1))
            bundles.append(lambda: qunit(2))
            bundles.append(lambda: qunit(3))
            # pop schedule: 2 units per superjob for the k/v chain (dead-
            # line superjob 2c), then 1 per superjob for the q units.
            pops = {}
            for idx in range(14):           # kunit/vunit c=1..7
                pops.setdefault(idx // 2, []).append(bundles[idx])
            pops[7] = [bundles[14]]         # qunit(1) well before block 1
            pops[8] = [bundles[15]]         # qunit(2) before block 2
            pops[9] = [bundles[16]]         # qunit(3) before block 3

            # ---- attention main loop ------------------------------------
            jobs = [(ci, tp) for ci in range(NBLK) for tp in range(TP)]

            def emit_scores(ci, tp):
                ps = pp_s.tile([128, 2, CHUNK], f32, tag="s",
                               name=f"s{ci}_{tp}")
                for j in range(2):
                    nc.tensor.matmul(
                        ps[:, j, :],
                        k8[:, :, bass.ds((2 * tp + j) * 128, 128)],
                        q8[:, :, bass.ds(ci * CHUNK, CHUNK)],
                        start=True, stop=True, perf_mode=DR,
                    )
                return ps

            pending = {jobs[0]: emit_scores(*jobs[0])}

            psum_o = None
            den = None
            for i, (ci, tp) in enumerate(jobs):
                if tp == 0:
                    psum_o = pp_o.tile([128, CHUNK], f32, tag="o",
                                       name=f"o{ci}")
                    den = pp_d.tile([128, CHUNK], f32, tag="d",
                                    name=f"d{ci}")

                # production bundles first: their drains queue ahead of
                # this superjob's exp on the same engines.
                for fn in pops.pop(i, []):
                    fn()

                ps_s = pending.pop((ci, tp))
                e_t = exps.tile([128, 2, CHUNK], i8, tag="e",
                                name=f"e{ci}_{tp}")
                ef = e_t.bitcast(f8)
                if busy["D"] + COST_EXP_DVE <= busy["A"] + COST_EXP_ACT:
                    busy["D"] += COST_EXP_DVE
                    nc.vector.tensor_scalar(
                        e_t, ps_s, B8, 0.0, op0=ADD, op1=MAX,
                    )
                else:
                    busy["A"] += COST_EXP_ACT
                    nc.scalar.activation(
                        ef, ps_s, mybir.ActivationFunctionType.Exp,
                        bias=bias_e[:, 0:1], scale=1.0 / A8,
                    )
                if i + 1 < len(jobs):
                    nj = jobs[i + 1]
                    pending[nj] = emit_scores(*nj)

                # P@V and denominator (DoubleRow, contraction 2x128/pass)
                nc.tensor.matmul(
                    psum_o, vT8[:, tp, :, :], ef,
                    start=(tp == 0), stop=(tp == TP - 1), perf_mode=DR,
                )
                nc.tensor.matmul(
                    den, ones8, ef,
                    start=(tp == 0), stop=(tp == TP - 1), perf_mode=DR,
                )

                # ---- block epilogue ----
                if tp == TP - 1:
                    cn = ci * CHUNK
                    rb = smalls.tile([128, CHUNK], f32, tag="rb",
                                     name=f"rb{ci}")
                    nc.vector.reciprocal_approx_fast(rb, den)
                    nc.vector.tensor_mul(
                        outN[:, bass.ds(cn, CHUNK)], psum_o, rb,
                    )
                    ps_y = pp_x.tile([128, CHUNK], f32, tag="x",
                                     name=f"psy{ci}")
                    nc.tensor.matmul(
                        ps_y, wo_sb, outN[:, bass.ds(cn, CHUNK)],
                        start=True, stop=True,
                    )
                    t2 = outs.tile([128, CHUNK], f32, tag="t2",
                                   name=f"t2{ci}")
                    nc.vector.scalar_tensor_tensor(
                        t2, ps_y, bo_sb, xh(cn, CHUNK),
                        op0=ADD, op1=ADD,
                    )
                    nc.sync.dma_start(
                        out=y_d[:, bass.ds(cn, CHUNK)], in_=t2
                    )
                    busy["D"] += COST_EPI_DVE

    nc.compile()
    _BUILT["nc"] = nc
    return nc


def _make_in_maps(inputs):
    x = np.asarray(inputs["x"], np.float32)
    wq = np.asarray(inputs["wq"], np.float32)
    bq = np.asarray(inputs["bq"], np.float32)
    wk = np.asarray(inputs["wk"], np.float32)
    wv = np.asarray(inputs["wv"], np.float32)
    bv = np.asarray(inputs["bv"], np.float32)
    wo = np.asarray(inputs["wo"], np.float32)
    bo = np.asarray(inputs["bo"], np.float32)
    gamma = float(np.asarray(inputs["gamma"], np.float32)[0])

    s = (1.0 / math.sqrt(C)) * A8
    wpack = np.ascontiguousarray(np.hstack([
        (wq * s).T, wk.T, wv.T, (wo * gamma).T,
    ]).astype(np.float16))
    bpack = np.ascontiguousarray(np.stack([
        bq * s, gamma * (wo @ bv + bo),
    ], axis=1).astype(np.float32))

    xf = x.reshape(B, C, N).astype(np.float16)
    in_maps = []
    for core in range(8):
        b, half = core // 2, core % 2
        own = xf[b][:, half * HALF:(half + 1) * HALF]
        oth = xf[b][:, (1 - half) * HALF:(2 - half) * HALF]
        in_maps.append({
            "x": np.ascontiguousarray(np.hstack([own, oth])),
            "wpack": wpack,
            "bpack": bpack,
        })
    return in_maps


def _gather(results):
    out = np.empty((B, C, N), np.float32)
    for core in range(8):
        b, half = core // 2, core % 2
        out[b][:, half * HALF:(half + 1) * HALF] = results[core]["y"]
    return out.reshape(B, C, W, H)


def run(inputs, trace=False):
    """Run on the 8 NeuronCores; returns (output, exec_time_ns_or_None)."""
    from concourse.bass_utils import run_bass_kernel_spmd

    nc = _build()
    in_maps = _make_in_maps(inputs)
    res = run_bass_kernel_spmd(nc, in_maps, core_ids=list(range(8)), trace=trace)
    return _gather(res.results), res.exec_time_ns


def kernel(**inputs):
    out, _ = run(inputs)
    return out


# revision 7
# speedup vs baseline: 1.7374x; 1.7374x over previous
"""Trainium2 Bass kernel for ConvSelfAttention (B=4, C=128, W=H=64).

Reference computation (per batch b, with N = W*H = 4096):
    q = wq @ x + bq ; k = wk @ x + bk ; v = wv @ x + bv        # [C, N]
    S[n, m] = (q[:, n] . k[:, m]) / sqrt(C)
    A = softmax(S, axis=m)                                     # [N, N]
    out[c, n] = sum_m v[c, m] A[n, m]
    y = wo @ out + bo
    result = gamma * y + x

Sharding: 8 cores = 4 batches x 2 halves of the attention-row dim n.
Each core holds full x[b] (for k, v) and computes out[:, n_slice].

v4 architecture (measured-model based; v1 fp16 baseline was 101us):
  - HW facts (probed): every matmul streams 1 output column/cycle at
    2.4GHz regardless of dtype (512 cols = 216ns); fp8 DoubleRow's win
    is doubled contraction (2x128) per pass, NOT faster columns; the PE
    p-state needs ~7us of gapless work to reach full clock and small
    (~100-200ns) waits don't reset it; Pool cannot touch PSUM and runs
    fp8 ALU ops 2-4x slow, so Pool does no math here.
  - Scores: plain fp16 matmuls (DoubleRow layouts would be the same
    speed and cost extra drains).  The exp scale A8 = 8/ln2 (and
    1/sqrt(C)) is folded into wq host-side so PSUM scores arrive
    pre-scaled for the fp8 exp bit trick.
  - exp (the drain wall): split between Scalar (native Exp, fp8 out,
    scale=1/A8, bias=-ln SC) and Vector (Schraudolph bit trick:
    int8(round(max(S' + B8, 0))) bitcast to fp8e4m3 ~ exp(S)/SC).
    Weights are normalized by their own approximate denominator so
    common-mode error cancels; the residual-dominated output tolerates
    the rest (tolerance 2e-2; attention ~12% of output norm).  A
    build-time greedy balancer assigns each exp to ACT or DVE around
    the static projection-drain copies.
  - P@V and the denominator run as fp8 DoubleRow matmuls over m-tile
    PAIRS (one 216ns pass contracts 2x128), halving their PE cost vs
    fp16.  The denominator ones-matmul broadcasts the per-column sums
    to all partitions for the epilogue reciprocal-multiply.
  - Loop: 4 blocks of 512 n-cols x 16 m-tile-pairs.  PSUM (8 banks):
    scores 2x[128,2,512] (4) + P@V accum (1) + den (1) + a 2-slot ring
    shared by projection drains -> out-proj (2).

Host-side folding: 1/sqrt(C) and A8 into wq, bq; bk dropped (cancels in
softmax); bv folded into bo via wo (softmax rows sum to 1); gamma into
wo and bo_eff.
"""

import math
import os
import sys

import numpy as np

if "/opt/trn_rl_repo" not in sys.path:
    sys.path.insert(0, "/opt/trn_rl_repo")

B, C, W, H = 4, 128, 64, 64
N = W * H            # 4096
HALF = N // 2        # 2048 n-columns per core
CHUNK = 512
MT = N // 128        # 32 m-tiles
TP = MT // 2         # 16 m-tile pairs
NBLK = HALF // CHUNK  # 4 n-blocks per core

LN2 = math.log(2.0)
A8 = 8.0 / LN2       # folded into wq host-side; scores arrive as A8*S
SC = 32.0            # weights are exp(S)/SC (fp8e4m3 range guard)
B8 = 8.0 * 7.0 - A8 * math.log(SC) - 0.22   # Schraudolph bias (round conv)

# build-time engine cost model (ns) for the greedy exp balancer
COST_EXP_ACT = 1113.0
COST_EXP_DVE = 1119.0
COST_COPY_ACT = 627.0
COST_COPY_DVE = 683.0
COST_EPI_DVE = 3 * 683.0

_BUILT = {}


def _build():
    if "nc" in _BUILT:
        return _BUILT["nc"]

    import concourse.bass as bass
    from concourse import bacc, mybir
    from concourse.tile import TileContext

    f32 = mybir.dt.float32
    f16 = mybir.dt.float16
    f8 = mybir.dt.float8e4
    i8 = mybir.dt.int8
    DR = mybir.MatmulPerfMode.DoubleRow
    ADD = mybir.AluOpType.add
    MAX = mybir.AluOpType.max

    nc = bacc.Bacc("TRN2", target_bir_lowering=False)

    x_d = nc.dram_tensor("x", [C, N], f16, kind="ExternalInput")
    wp_d = nc.dram_tensor("wpack", [C, 4 * C], f16, kind="ExternalInput")
    bp_d = nc.dram_tensor("bpack", [C, 2], f32, kind="ExternalInput")
    y_d = nc.dram_tensor("y", [C, HALF], f32, kind="ExternalOutput")

    busy = {"A": 0.0, "D": 0.0}   # projected ACT / DVE busy-ns

    with TileContext(nc) as tc:
        with (
            tc.tile_pool(name="consts", bufs=1) as consts,
            tc.tile_pool(name="bigs", bufs=1) as bigs,
            tc.tile_pool(name="exps", bufs=6) as exps,
            tc.tile_pool(name="smalls", bufs=2) as smalls,
            tc.tile_pool(name="outs", bufs=3) as outs,
            tc.tile_pool(name="xs", bufs=4) as xs,
            tc.tile_pool(name="ps_s", bufs=2, space="PSUM") as pp_s,
            tc.tile_pool(name="ps_o", bufs=1, space="PSUM") as pp_o,
            tc.tile_pool(name="ps_d", bufs=1, space="PSUM") as pp_d,
            tc.tile_pool(name="ps_x", bufs=2, space="PSUM") as pp_x,
        ):
            # ---- constants & inputs -------------------------------------
            wp_sb = consts.tile([C, 4 * C], f16, tag="wp")
            bp_sb = consts.tile([C, 2], f32, tag="bp")
            ones8 = consts.tile([128, 2, C], f8, tag="on8")
            bias_e = consts.tile([128, 1], f32, tag="be")
            dummy = consts.tile([C, 1], f16, tag="dmy")

            wq_sb = wp_sb[:, 0:C]          # s*A8 folded
            wk_sb = wp_sb[:, C:2 * C]
            wv_sb = wp_sb[:, 2 * C:3 * C]
            wo_sb = wp_sb[:, 3 * C:4 * C]  # gamma folded
            bq_sb = bp_sb[:, 0:1]
            bo_sb = bp_sb[:, 1:2]

            # Warm the ACT exp table while DMAs are in flight.
            nc.vector.memset(bias_e, -math.log(SC))
            nc.vector.memset(dummy, 0.0)
            nc.scalar.activation(
                dummy, dummy, mybir.ActivationFunctionType.Exp,
                bias=bias_e[:, 0:1],
            )
            nc.vector.memset(ones8, 1.0)

            k16 = bigs.tile([C, N], f16, tag="k16")
            q16 = bigs.tile([C, HALF], f16, tag="q16")
            vT8 = bigs.tile([128, TP, 2, C], f8, tag="vT8")
            outN = bigs.tile([C, HALF], f16, tag="outN")

            # x pieces: 4 x 1024 cols
            nc.sync.dma_start(out=wp_sb, in_=wp_d[:, :])
            nc.gpsimd.dma_start(out=bp_sb, in_=bp_d[:, :])
            piece = []
            for pi in range(4):
                xp = xs.tile([C, 1024], f16, tag="xp", name=f"xp{pi}")
                eng = nc.scalar if pi < 2 else nc.sync
                eng.dma_start(out=xp, in_=x_d[:, bass.ds(1024 * pi, 1024)])
                piece.append(xp)

            def xh(cn, w):
                pi, off = divmod(cn, 1024)
                assert off + w <= 1024
                return piece[pi][:, bass.ds(off, w)]

            # ---- production units (512 m-cols each) ---------------------
            def kunit(c):
                ps = pp_x.tile([128, CHUNK], f32, tag="x", name=f"psk{c}")
                nc.tensor.matmul(ps, wk_sb, xh(c * CHUNK, CHUNK),
                                 start=True, stop=True)
                nc.scalar.activation(
                    k16[:, bass.ds(c * CHUNK, CHUNK)], ps,
                    mybir.ActivationFunctionType.Copy,
                )
                busy["A"] += COST_COPY_ACT

            def qunit(c):
                ps = pp_x.tile([128, CHUNK], f32, tag="x", name=f"psq{c}")
                nc.tensor.matmul(ps, wq_sb, xh(c * CHUNK, CHUNK),
                                 start=True, stop=True)
                nc.vector.tensor_scalar_add(
                    q16[:, bass.ds(c * CHUNK, CHUNK)], ps, bq_sb,
                )
                busy["D"] += COST_COPY_DVE

            def vunit(g):
                ps = pp_x.tile([128, CHUNK], f32, tag="x", name=f"psv{g}")
                for tt in range(4):
                    nc.tensor.matmul(
                        ps[:, bass.ts(tt, 128)],
                        xh(g * CHUNK + tt * 128, 128),
                        wv_sb, start=True, stop=True,
                    )
                nc.vector.tensor_copy(
                    vT8[:, bass.ds(2 * g, 2), :, :],
                    ps.rearrange("p (t c) -> p t c", c=C),
                )
                busy["D"] += COST_COPY_DVE

            # prefetch the units the first superjobs need
            kunit(0)
            qunit(0)
            vunit(0)
            # k/v chunk c is consumed from superjob (=tp in block 0) 2c;
            # q chunk c from block c (superjob 16c).
            pops = {}
            for c in range(1, 8):
                pops.setdefault(max(0, 2 * c - 3), []).append(
                    lambda c=c: kunit(c))
                pops.setdefault(max(0, 2 * c - 2), []).append(
                    lambda g=c: vunit(g))
            pops.setdefault(3, []).append(lambda: qunit(1))
            pops.setdefault(5, []).append(lambda: qunit(2))
            pops.setdefault(7, []).append(lambda: qunit(3))

            # ---- attention main loop ------------------------------------
            jobs = [(ci, tp) for ci in range(NBLK) for tp in range(TP)]

            def emit_scores(ci, tp):
                ps = pp_s.tile([128, 2, CHUNK], f32, tag="s",
                               name=f"s{ci}_{tp}")
                for j in range(2):
                    nc.tensor.matmul(
                        ps[:, j, :],
                        k16[:, bass.ds((2 * tp + j) * 128, 128)],
                        q16[:, bass.ds(ci * CHUNK, CHUNK)],
                        start=True, stop=True,
                    )
                return ps

            pending = {jobs[0]: emit_scores(*jobs[0])}

            def epilogue2(ci, psum_o, rb):
                # deferred: out-projection + bias/residual + store.
                cn = ci * CHUNK
                nc.vector.tensor_mul(
                    outN[:, bass.ds(cn, CHUNK)], psum_o, rb,
                )
                ps_y = pp_x.tile([128, CHUNK], f32, tag="x",
                                 name=f"psy{ci}")
                nc.tensor.matmul(
                    ps_y, wo_sb, outN[:, bass.ds(cn, CHUNK)],
                    start=True, stop=True,
                )
                t2 = outs.tile([128, CHUNK], f32, tag="t2",
                               name=f"t2{ci}")
                nc.vector.scalar_tensor_tensor(
                    t2, ps_y, bo_sb, xh(cn, CHUNK),
                    op0=ADD, op1=ADD,
                )
                nc.sync.dma_start(out=y_d[:, bass.ds(cn, CHUNK)], in_=t2)
                busy["D"] += COST_EPI_DVE

            psum_o = None
            den = None
            for i, (ci, tp) in enumerate(jobs):
                if tp == 0:
                    psum_o = pp_o.tile([128, CHUNK], f32, tag="o",
                                       name=f"o{ci}")
                    den = pp_d.tile([128, CHUNK], f32, tag="d",
                                    name=f"d{ci}")

                # production bundles first: their drains queue ahead of
                # this superjob's exp on the same engines.
                for fn in pops.pop(i, []):
                    fn()

                ps_s = pending.pop((ci, tp))
                e_t = exps.tile([128, 2, CHUNK], i8, tag="e",
                                name=f"e{ci}_{tp}")
                ef = e_t.bitcast(f8)
                if busy["D"] + COST_EXP_DVE <= busy["A"] + COST_EXP_ACT:
                    busy["D"] += COST_EXP_DVE
                    nc.vector.tensor_scalar(
                        e_t, ps_s, B8, 0.0, op0=ADD, op1=MAX,
                    )
                else:
                    busy["A"] += COST_EXP_ACT
                    nc.scalar.activation(
                        ef, ps_s, mybir.ActivationFunctionType.Exp,
                        bias=bias_e[:, 0:1], scale=1.0 / A8,
                    )
                if i + 1 < len(jobs):
                    nj = jobs[i + 1]
                    pending[nj] = emit_scores(*nj)

                # P@V and denominator (DoubleRow, contraction 2x128/pass)
                nc.tensor.matmul(
                    psum_o, vT8[:, tp, :, :], ef,
                    start=(tp == 0), stop=(tp == TP - 1), perf_mode=DR,
                )
                nc.tensor.matmul(
                    den, ones8, ef,
                    start=(tp == 0), stop=(tp == TP - 1), perf_mode=DR,
                )

                # ---- block epilogue (reciprocal now; the rest deferred
                # into the next block so out-proj doesn't stall PE) ----
                if tp == TP - 1:
                    rb = smalls.tile([128, CHUNK], f32, tag="rb",
                                     name=f"rb{ci}")
                    nc.vector.reciprocal_approx_fast(rb, den)
                    if ci < NBLK - 1:
                        pops.setdefault(i + 3, []).append(
                            lambda ci=ci, po=psum_o, rb=rb:
                            epilogue2(ci, po, rb))
                    else:
                        epilogue2(ci, psum_o, rb)

    nc.compile()
    _BUILT["nc"] = nc
    return nc


def _make_in_maps(inputs):
    x = np.asarray(inputs["x"], np.float32)
    wq = np.asarray(inputs["wq"], np.float32)
    bq = np.asarray(inputs["bq"], np.float32)
    wk = np.asarray(inputs["wk"], np.float32)
    wv = np.asarray(inputs["wv"], np.float32)
    bv = np.asarray(inputs["bv"], np.float32)
    wo = np.asarray(inputs["wo"], np.float32)
    bo = np.asarray(inputs["bo"], np.float32)
    gamma = float(np.asarray(inputs["gamma"], np.float32)[0])

    s = (1.0 / math.sqrt(C)) * A8
    wpack = np.ascontiguousarray(np.hstack([
        (wq * s).T, wk.T, wv.T, (wo * gamma).T,
    ]).astype(np.float16))
    bpack = np.ascontiguousarray(np.stack([
        bq * s, gamma * (wo @ bv + bo),
    ], axis=1).astype(np.float32))

    xf = x.reshape(B, C, N).astype(np.float16)
    in_maps = []
    for core in range(8):
        b, half = core // 2, core % 2
        own = xf[b][:, half * HALF:(half + 1) * HALF]
        oth = xf[b][:, (1 - half) * HALF:(2 - half) * HALF]
        in_maps.append({
            "x": np.ascontiguousarray(np.hstack([own, oth])),
            "wpack": wpack,
            "bpack": bpack,
        })
    return in_maps


def _gather(results):
    out = np.empty((B, C, N), np.float32)
    for core in range(8):
        b, half = core // 2, core % 2
        out[b][:, half * HALF:(half + 1) * HALF] = results[core]["y"]
    return out.reshape(B, C, W, H)


def run(inputs, trace=False):
    """Run on the 8 NeuronCores; returns (output, exec_time_ns_or_None)."""
    from concourse.bass_utils import run_bass_kernel_spmd

    nc = _build()
    in_maps = _make_in_maps(inputs)
    res = run_bass_kernel_spmd(nc, in_maps, core_ids=list(range(8)), trace=trace)
    return _gather(res.results), res.exec_time_ns


def kernel(**inputs):
    out, _ = run(inputs)
    return out


# revision 10
# speedup vs baseline: 1.9194x; 1.1048x over previous
"""Trainium2 Bass kernel for ConvSelfAttention (B=4, C=128, W=H=64).

Reference computation (per batch b, with N = W*H = 4096):
    q = wq @ x + bq ; k = wk @ x + bk ; v = wv @ x + bv        # [C, N]
    S[n, m] = (q[:, n] . k[:, m]) / sqrt(C)
    A = softmax(S, axis=m)                                     # [N, N]
    out[c, n] = sum_m v[c, m] A[n, m]
    y = wo @ out + bo
    result = gamma * y + x

Sharding: 8 cores = 4 batches x 2 halves of the attention-row dim n.
Each core holds full x[b] (for k, v) and computes out[:, n_slice].

v4 architecture (measured-model based; v1 fp16 baseline was 101us):
  - HW facts (probed): every matmul streams 1 output column/cycle at
    2.4GHz regardless of dtype (512 cols = 216ns); fp8 DoubleRow's win
    is doubled contraction (2x128) per pass, NOT faster columns; the PE
    p-state needs ~7us of gapless work to reach full clock and small
    (~100-200ns) waits don't reset it; Pool cannot touch PSUM and runs
    fp8 ALU ops 2-4x slow, so Pool does no math here.
  - Scores: plain fp16 matmuls (DoubleRow layouts would be the same
    speed and cost extra drains).  The exp scale A8 = 8/ln2 (and
    1/sqrt(C)) is folded into wq host-side so PSUM scores arrive
    pre-scaled for the fp8 exp bit trick.
  - exp (the drain wall): split between Scalar (native Exp, fp8 out,
    scale=1/A8, bias=-ln SC) and Vector (Schraudolph bit trick:
    int8(round(max(S' + B8, 0))) bitcast to fp8e4m3 ~ exp(S)/SC).
    Weights are normalized by their own approximate denominator so
    common-mode error cancels; the residual-dominated output tolerates
    the rest (tolerance 2e-2; attention ~12% of output norm).  A
    build-time greedy balancer assigns each exp to ACT or DVE around
    the static projection-drain copies.
  - P@V and the denominator run as fp8 DoubleRow matmuls over m-tile
    PAIRS (one 216ns pass contracts 2x128), halving their PE cost vs
    fp16.  The denominator ones-matmul broadcasts the per-column sums
    to all partitions for the epilogue reciprocal-multiply.
  - Loop: 4 blocks of 512 n-cols x 16 m-tile-pairs.  PSUM (8 banks):
    scores 2x[128,2,512] (4) + P@V accum (1) + den (1) + a 2-slot ring
    shared by projection drains -> out-proj (2).

Host-side folding: 1/sqrt(C) and A8 into wq, bq; bk dropped (cancels in
softmax); bv folded into bo via wo (softmax rows sum to 1); gamma into
wo and bo_eff.
"""

import math
import os
import sys

import numpy as np

if "/opt/trn_rl_repo" not in sys.path:
    sys.path.insert(0, "/opt/trn_rl_repo")

B, C, W, H = 4, 128, 64, 64
N = W * H            # 4096
HALF = N // 2        # 2048 n-columns per core
CHUNK = 512
MT = N // 128        # 32 m-tiles
TP = MT // 2         # 16 m-tile pairs
NBLK = HALF // CHUNK  # 4 n-blocks per core

LN2 = math.log(2.0)
A8 = 8.0 / LN2       # folded into wq host-side; scores arrive as A8*S
SC = 32.0            # weights are exp(S)/SC (fp8e4m3 range guard)
B8 = 8.0 * 7.0 - A8 * math.log(SC) - 0.22   # Schraudolph bias (round conv)

# build-time engine cost model (ns) for the greedy exp balancer
COST_EXP_ACT = 1113.0
COST_EXP_DVE = 1119.0
COST_COPY_ACT = 627.0
COST_COPY_DVE = 683.0
COST_EPI_DVE = 3 * 683.0

_BUILT = {}


def _build():
    if "nc" in _BUILT:
        return _BUILT["nc"]

    import concourse.bass as bass
    from concourse import bacc, mybir
    from concourse.tile import TileContext

    f32 = mybir.dt.float32
    f16 = mybir.dt.float16
    f8 = mybir.dt.float8e4
    i8 = mybir.dt.int8
    DR = mybir.MatmulPerfMode.DoubleRow
    ADD = mybir.AluOpType.add
    MAX = mybir.AluOpType.max

    nc = bacc.Bacc("TRN2", target_bir_lowering=False)

    x_d = nc.dram_tensor("x", [C, N], f16, kind="ExternalInput")
    wp_d = nc.dram_tensor("wpack", [C, 4 * C], f16, kind="ExternalInput")
    bp_d = nc.dram_tensor("bpack", [C, 2], f32, kind="ExternalInput")
    y_d = nc.dram_tensor("y", [C, HALF], f32, kind="ExternalOutput")

    busy = {"A": 0.0, "D": 0.0}   # projected ACT / DVE busy-ns

    with TileContext(nc) as tc:
        with (
            tc.tile_pool(name="consts", bufs=1) as consts,
            tc.tile_pool(name="bigs", bufs=1) as bigs,
            tc.tile_pool(name="exps", bufs=6) as exps,
            tc.tile_pool(name="smalls", bufs=2) as smalls,
            tc.tile_pool(name="outs", bufs=3) as outs,
            tc.tile_pool(name="xs", bufs=4) as xs,
            tc.tile_pool(name="ps_s", bufs=2, space="PSUM") as pp_s,
            tc.tile_pool(name="ps_o", bufs=1, space="PSUM") as pp_o,
            tc.tile_pool(name="ps_d", bufs=1, space="PSUM") as pp_d,
            tc.tile_pool(name="ps_x", bufs=2, space="PSUM") as pp_x,
        ):
            # ---- constants & inputs -------------------------------------
            wp_sb = consts.tile([C, 4 * C], f16, tag="wp")
            bp_sb = consts.tile([C, 2], f32, tag="bp")
            ones8 = consts.tile([128, 2, C], f8, tag="on8")
            bias_e = consts.tile([128, 1], f32, tag="be")
            dummy = consts.tile([C, 1], f16, tag="dmy")

            wq_sb = wp_sb[:, 0:C]          # s*A8 folded
            wk_sb = wp_sb[:, C:2 * C]
            wv_sb = wp_sb[:, 2 * C:3 * C]
            wo_sb = wp_sb[:, 3 * C:4 * C]  # gamma folded
            bq_sb = bp_sb[:, 0:1]
            bo_sb = bp_sb[:, 1:2]

            # Warm the ACT exp table while DMAs are in flight.
            nc.vector.memset(bias_e, -math.log(SC))
            nc.vector.memset(dummy, 0.0)
            nc.scalar.activation(
                dummy, dummy, mybir.ActivationFunctionType.Exp,
                bias=bias_e[:, 0:1],
            )
            nc.vector.memset(ones8, 1.0)

            k16 = bigs.tile([C, N], f16, tag="k16")
            q16 = bigs.tile([C, HALF], f16, tag="q16")
            vT8 = bigs.tile([128, TP, 2, C], f8, tag="vT8")
            outN = bigs.tile([C, HALF], f16, tag="outN")

            # x pieces: 4 x 1024 cols
            nc.sync.dma_start(out=wp_sb, in_=wp_d[:, :])
            nc.gpsimd.dma_start(out=bp_sb, in_=bp_d[:, :])
            piece = []
            for pi in range(4):
                xp = xs.tile([C, 1024], f16, tag="xp", name=f"xp{pi}")
                eng = nc.scalar if pi < 2 else nc.sync
                eng.dma_start(out=xp, in_=x_d[:, bass.ds(1024 * pi, 1024)])
                piece.append(xp)

            def xh(cn, w):
                pi, off = divmod(cn, 1024)
                assert off + w <= 1024
                return piece[pi][:, bass.ds(off, w)]

            # ---- production units (512 m-cols each) ---------------------
            def kunit(c):
                ps = pp_x.tile([128, CHUNK], f32, tag="x", name=f"psk{c}")
                nc.tensor.matmul(ps, wk_sb, xh(c * CHUNK, CHUNK),
                                 start=True, stop=True)
                nc.scalar.activation(
                    k16[:, bass.ds(c * CHUNK, CHUNK)], ps,
                    mybir.ActivationFunctionType.Copy,
                )
                busy["A"] += COST_COPY_ACT

            def qunit(c):
                ps = pp_x.tile([128, CHUNK], f32, tag="x", name=f"psq{c}")
                nc.tensor.matmul(ps, wq_sb, xh(c * CHUNK, CHUNK),
                                 start=True, stop=True)
                nc.vector.tensor_scalar_add(
                    q16[:, bass.ds(c * CHUNK, CHUNK)], ps, bq_sb,
                )
                busy["D"] += COST_COPY_DVE

            def vunit(g):
                ps = pp_x.tile([128, CHUNK], f32, tag="x", name=f"psv{g}")
                for tt in range(4):
                    nc.tensor.matmul(
                        ps[:, bass.ts(tt, 128)],
                        xh(g * CHUNK + tt * 128, 128),
                        wv_sb, start=True, stop=True,
                    )
                nc.vector.tensor_copy(
                    vT8[:, bass.ds(2 * g, 2), :, :],
                    ps.rearrange("p (t c) -> p t c", c=C),
                )
                busy["D"] += COST_COPY_DVE

            # prefetch the units the first superjobs need
            kunit(0)
            qunit(0)
            vunit(0)
            # k/v chunk c is consumed from superjob (=tp in block 0) 2c;
            # q chunk c from block c (superjob 16c).
            pops = {}
            for c in range(1, 8):
                pops.setdefault(max(0, 2 * c - 3), []).append(
                    lambda c=c: kunit(c))
                pops.setdefault(max(0, 2 * c - 2), []).append(
                    lambda g=c: vunit(g))
            pops.setdefault(3, []).append(lambda: qunit(1))
            pops.setdefault(5, []).append(lambda: qunit(2))
            pops.setdefault(7, []).append(lambda: qunit(3))

            # ---- attention main loop ------------------------------------
            jobs = [(ci, tp) for ci in range(NBLK) for tp in range(TP)]

            def emit_scores(ci, tp):
                ps = pp_s.tile([128, 2, CHUNK], f32, tag="s",
                               name=f"s{ci}_{tp}")
                for j in range(2):
                    nc.tensor.matmul(
                        ps[:, j, :],
                        k16[:, bass.ds((2 * tp + j) * 128, 128)],
                        q16[:, bass.ds(ci * CHUNK, CHUNK)],
                        start=True, stop=True,
                    )
                return ps

            pending = {jobs[0]: emit_scores(*jobs[0])}

            def epilogue2(ci, psum_o, rb):
                # deferred: out-projection + bias/residual + store.
                cn = ci * CHUNK
                nc.vector.tensor_mul(
                    outN[:, bass.ds(cn, CHUNK)], psum_o, rb,
                )
                ps_y = pp_x.tile([128, CHUNK], f32, tag="x",
                                 name=f"psy{ci}")
                nc.tensor.matmul(
                    ps_y, wo_sb, outN[:, bass.ds(cn, CHUNK)],
                    start=True, stop=True,
                )
                t2 = outs.tile([128, CHUNK], f32, tag="t2",
                               name=f"t2{ci}")
                nc.vector.scalar_tensor_tensor(
                    t2, ps_y, bo_sb, xh(cn, CHUNK),
                    op0=ADD, op1=ADD,
                )
                nc.sync.dma_start(out=y_d[:, bass.ds(cn, CHUNK)], in_=t2)
                busy["D"] += COST_EPI_DVE

            psum_o = None
            den = None
            delayed = []
            for i, (ci, tp) in enumerate(jobs):
                if tp == 0:
                    psum_o = pp_o.tile([128, CHUNK], f32, tag="o",
                                       name=f"o{ci}")
                    den = pp_d.tile([128, CHUNK], f32, tag="d",
                                    name=f"d{ci}")

                # production bundles first: their drains queue ahead of
                # this superjob's exp on the same engines.
                for fn in pops.pop(i, []):
                    fn()

                ps_s = pending.pop((ci, tp))
                e_t = exps.tile([128, 2, CHUNK], i8, tag="e",
                                name=f"e{ci}_{tp}")
                ef = e_t.bitcast(f8)
                if busy["D"] + COST_EXP_DVE <= busy["A"] + COST_EXP_ACT:
                    busy["D"] += COST_EXP_DVE
                    nc.vector.tensor_scalar(
                        e_t, ps_s, B8, 0.0, op0=ADD, op1=MAX,
                    )
                else:
                    busy["A"] += COST_EXP_ACT
                    nc.scalar.activation(
                        ef, ps_s, mybir.ActivationFunctionType.Exp,
                        bias=bias_e[:, 0:1], scale=1.0 / A8,
                    )
                if i + 1 < len(jobs):
                    nj = jobs[i + 1]
                    pending[nj] = emit_scores(*nj)

                # P@V and denominator are DELAYED one superjob: by the
                # time they reach the PE their exp is already complete,
                # so the next superjob's scores (ahead of them in the
                # in-order PE queue) never stall the exp engines.
                def pvden(ci=ci, tp=tp, ef=ef, po=psum_o, dn=den, i=i):
                    nc.tensor.matmul(
                        po, vT8[:, tp, :, :], ef,
                        start=(tp == 0), stop=(tp == TP - 1),
                        perf_mode=DR,
                    )
                    nc.tensor.matmul(
                        dn, ones8, ef,
                        start=(tp == 0), stop=(tp == TP - 1),
                        perf_mode=DR,
                    )
                    # block epilogue: reciprocal now; out-proj deferred
                    # further so it doesn't stall PE either.
                    if tp == TP - 1:
                        rb = smalls.tile([128, CHUNK], f32, tag="rb",
                                         name=f"rb{ci}")
                        nc.vector.reciprocal_approx_fast(rb, dn)
                        if ci < NBLK - 1:
                            pops.setdefault(i + 3, []).append(
                                lambda: epilogue2(ci, po, rb))
                        else:
                            delayed.append(lambda: epilogue2(ci, po, rb))

                delayed.append(pvden)
                if len(delayed) > 1:
                    delayed.pop(0)()

            while delayed:
                delayed.pop(0)()

    nc.compile()
    _BUILT["nc"] = nc
    return nc


def _make_in_maps(inputs):
    x = np.asarray(inputs["x"], np.float32)
    wq = np.asarray(inputs["wq"], np.float32)
    bq = np.asarray(inputs["bq"], np.float32)
    wk = np.asarray(inputs["wk"], np.float32)
    wv = np.asarray(inputs["wv"], np.float32)
    bv = np.asarray(inputs["bv"], np.float32)
    wo = np.asarray(inputs["wo"], np.float32)
    bo = np.asarray(inputs["bo"], np.float32)
    gamma = float(np.asarray(inputs["gamma"], np.float32)[0])

    s = (1.0 / math.sqrt(C)) * A8
    wpack = np.ascontiguousarray(np.hstack([
        (wq * s).T, wk.T, wv.T, (wo * gamma).T,
    ]).astype(np.float16))
    bpack = np.ascontiguousarray(np.stack([
        bq * s, gamma * (wo @ bv + bo),
    ], axis=1).astype(np.float32))

    xf = x.reshape(B, C, N).astype(np.float16)
    in_maps = []
    for core in range(8):
        b, half = core // 2, core % 2
        own = xf[b][:, half * HALF:(half + 1) * HALF]
        oth = xf[b][:, (1 - half) * HALF:(2 - half) * HALF]
        in_maps.append({
            "x": np.ascontiguousarray(np.hstack([own, oth])),
            "wpack": wpack,
            "bpack": bpack,
        })
    return in_maps


def _gather(results):
    out = np.empty((B, C, N), np.float32)
    for core in range(8):
        b, half = core // 2, core % 2
        out[b][:, half * HALF:(half + 1) * HALF] = results[core]["y"]
    return out.reshape(B, C, W, H)


def run(inputs, trace=False):
    """Run on the 8 NeuronCores; returns (output, exec_time_ns_or_None)."""
    from concourse.bass_utils import run_bass_kernel_spmd

    nc = _build()
    in_maps = _make_in_maps(inputs)
    res = run_bass_kernel_spmd(nc, in_maps, core_ids=list(range(8)), trace=trace)
    return _gather(res.results), res.exec_time_ns


def kernel(**inputs):
    out, _ = run(inputs)
    return out
